# revision 18
# baseline (speedup 1.0000x reference)
"""BFP-quantized 3x3 conv (nn_BFConv2d) on 8 TRN2 NeuronCores.

Two-invocation structure (the BFP group grid is global over the flat
tensor, so each sample's quantized slab starts at a per-(sample,core)
phase pre = start mod 36; re-chunking by pre must happen host-side,
forcing quantize and conv into separate NEFF executions):

  inv1 QUANT: per core, 4 samples. Host supplies bf16-cast (RNE)
    group-aligned windows [128, 6300]. GpSimd computes per-group abs-max
    (groups are bf16 so the max is exact and its exponent equals the f32
    exponent), Vector applies the magic-number snap
        q = (x + M) - M,  M = 1.5 * 2^16 * exp2_bits(absmax)
    (round-half-even onto the BFP lattice; lattice points are <=9
    significant bits so bf16 holds q exactly). DMA in/out ~12.9MB/core.

  inv2 CONV: rho-paired matmul scheme at 75% PE utilization: PSUM
    partitions = 64 outch x 2 output-row-parity, K = 64 inch x 2
    input-row-offsets (dense 128x128 weights, host-built), 6 matmuls
    (2 K-chunks x 3 dw taps) of N=452 per 8-row tile. Input is a
    host-built duplicated padded layout [128, 12884] bf16 per sample
    (partitions 64-127 = same image shifted one row) with 113-strided
    rows sharing single zero pad columns. Output written bf16 in a
    partition-major layout, host de-interleaves rows and casts to f32.
"""

import os
import sys
from contextlib import ExitStack

import numpy as np

sys.path.insert(0, "/opt/trn_rl_repo")

import ml_dtypes  # noqa: E402
import concourse.bacc as bacc  # noqa: E402
import concourse.mybir as mybir  # noqa: E402
import concourse.tile as tile  # noqa: E402

F32 = mybir.dt.float32
BF16 = mybir.dt.bfloat16
I16 = mybir.dt.int16
NPBF16 = ml_dtypes.bfloat16

N_CORES = 8
B = 32                      # batch
C = 64                      # channels (in == out)
H = W = 112
SAMPLE = C * H * W          # 802816 elems per sample
GS = 36                     # BFP group size
GPP = 175                   # groups per partition in the quantize window
QCOLS = GPP * GS            # 6300
QWIN = 128 * QCOLS          # 806400 elems: covers a sample + phase slack
T = 12884                   # conv tile cols: 1 guard + 114*113 + 1 spare
MAGIC_MUL = 98304.0         # 1.5 * 2^16: exp2(e)*this == 1.5*2^23*2^(e-7)

_cache = {}
last_exec_ns = {}
last_results = {}


def _ensure_snap_op():
    """Register a custom DVE op BFP_SNAP_ANT: out = (in0 + in1) - in1."""
    import concourse.dve_ops as dops
    if getattr(dops, "_BFP_SNAP_ANT", None) is not None:
        return dops._BFP_SNAP_ANT
    from concourse.dve_spec import Spec, Src0, Src1, lower as spec_lower
    from concourse.dve_uop import DveOpSpec

    def _snap_ref(in0, in1, s0, s1, imm2):
        a = in0.astype(np.float32)
        b = np.broadcast_to(in1.astype(np.float32), in1.shape).reshape(a.shape)
        return (a + b) - b

    spec = Spec(body=(Src0 + Src1) - Src1, reference=_snap_ref)
    op = dops.DveOp("BFP_SNAP_ANT", spec, subdim=False, uops_sha={})
    idx = max(dops._SUB_OPCODE_FOR_NAME.values()) + 1
    assert idx < 0x20
    dops.OPS.append(op)
    dops.CUSTOM_DVE_SPECS["BFP_SNAP_ANT"] = spec
    dops._SUB_OPCODE_FOR_NAME["BFP_SNAP_ANT"] = idx
    for ver in ("v3", "v4"):
        try:
            s = DveOpSpec(name=op.name, opcode=idx,
                          uops=spec_lower(spec, ver=ver), rd1_en=True)
            op.uops_sha[ver] = s.sha(ver)
        except Exception:
            pass
    dops._BFP_SNAP_ANT = op
    return op


def _trace_enabled():
    return os.environ.get("BFP_TRACE") == "1"


def _install_trace_shim():
    """Provide antenv.axon_hooks (NTFF profiling hook) if the image lacks it."""
    import types
    import ctypes
    import contextlib
    try:
        from antenv.axon_hooks import get_axon_ntff_profile_hook  # noqa: F401
        return
    except ImportError:
        pass
    so_path = "/opt/axon/libaxon_pjrt.so"
    if not os.path.exists(so_path):
        return
    lib = ctypes.CDLL(so_path)
    if not hasattr(lib, "axon_start_nrt_profile"):
        return
    lib.axon_start_nrt_profile.argtypes = [ctypes.POINTER(ctypes.c_int64),
                                           ctypes.c_size_t]
    lib.axon_start_nrt_profile.restype = ctypes.c_int64
    lib.axon_stop_nrt_profile.argtypes = [ctypes.c_char_p]
    lib.axon_stop_nrt_profile.restype = ctypes.c_int64

    @contextlib.contextmanager
    def _hook(output_dir, device_ids):
        import jax
        jax.devices()
        if device_ids:
            ids = (ctypes.c_int64 * len(device_ids))(*device_ids)
            rc = lib.axon_start_nrt_profile(ids, len(device_ids))
        else:
            rc = lib.axon_start_nrt_profile(None, 0)
        if rc != 0:
            raise RuntimeError(f"axon_start_nrt_profile rc={rc}")
        try:
            yield
        finally:
            n = lib.axon_stop_nrt_profile(str(output_dir).encode())
            print(f"profile: {n} ntff file(s) -> {output_dir}", file=sys.stderr)

    mod = types.ModuleType("antenv.axon_hooks")
    state = {"hook": _hook}
    mod.get_axon_ntff_profile_hook = lambda: state["hook"]
    mod.set_axon_ntff_profile_hook = lambda h: state.update(hook=h)
    sys.modules["antenv.axon_hooks"] = mod
    import antenv
    antenv.axon_hooks = mod
    from concourse import bass_utils as bu
    bu.upload_artifacts = lambda d: str(d)  # no egress from this container


ICOLS = 36 * 176            # 6336: interleaved window, col = s*176 + g


def build_quant():
    """Interleaved-layout quant: host delivers windows as [s:36, g:176]
    (col = s*176 + g, g=175 is zero pad) so the per-group abs-max becomes a
    cascade of contiguous builtin tensor_tensor max ops (2x-eligible), with
    abs on ScalarE and the magic snap as a custom DVE op."""
    snap = _ensure_snap_op()
    nc = bacc.Bacc(None)
    xin = nc.declare_dram_parameter("xin", [4, 128, ICOLS], BF16, isOutput=False)
    qx = nc.declare_dram_parameter("qx", [4, 128, ICOLS], BF16, isOutput=True)

    MAX = mybir.AluOpType.max
    with tile.TileContext(nc) as tc:
        with ExitStack() as ctx:
            xpool = ctx.enter_context(tc.tile_pool(name="xp", bufs=3))
            apool = ctx.enter_context(tc.tile_pool(name="ap", bufs=2))
            qpool = ctx.enter_context(tc.tile_pool(name="qp", bufs=2))
            spool = ctx.enter_context(tc.tile_pool(name="small", bufs=2))

            def absmax_phase(j):
                xb = xpool.tile([128, ICOLS], BF16, tag="xb", name="xb")
                nc.sync.dma_start(xb[:], xin[j])
                xa = apool.tile([128, ICOLS], BF16, tag="xa", name="xa")
                nc.scalar.activation(xa[:], xb[:],
                                     mybir.ActivationFunctionType.Abs)
                u1 = spool.tile([128, 2816], BF16, tag="u1", name="u1")
                nc.vector.tensor_tensor(u1[:], xa[:, 0:2816], xa[:, 2816:5632], MAX)
                u2 = spool.tile([128, 1408], BF16, tag="u2", name="u2")
                nc.vector.tensor_tensor(u2[:], u1[:, 0:1408], u1[:, 1408:2816], MAX)
                u3 = spool.tile([128, 704], BF16, tag="u3", name="u3")
                nc.vector.tensor_tensor(u3[:], u2[:, 0:704], u2[:, 704:1408], MAX)
                u4 = spool.tile([128, 704], BF16, tag="u4", name="u4")
                nc.vector.tensor_tensor(u4[:], u3[:, 0:704], xa[:, 5632:6336], MAX)
                u5 = spool.tile([128, 352], BF16, tag="u5", name="u5")
                nc.vector.tensor_tensor(u5[:], u4[:, 0:352], u4[:, 352:704], MAX)
                m = spool.tile([128, 176], BF16, tag="m", name="m")
                nc.vector.tensor_tensor(m[:], u5[:, 0:176], u5[:, 176:352], MAX)
                mi = spool.tile([128, 176], I16, tag="mi", name="mi")
                nc.vector.tensor_scalar(mi[:], m[:].bitcast(I16), 0x7F80, None,
                                        op0=mybir.AluOpType.bitwise_and)
                mf = spool.tile([128, 176], BF16, tag="mf", name="mf")
                nc.vector.tensor_scalar(mf[:], mi[:].bitcast(BF16), MAGIC_MUL,
                                        None, op0=mybir.AluOpType.mult)
                return xb, mf

            def snap_phase(j, xb, mf):
                # two half-snaps so the first half's store overlaps the second
                q = qpool.tile([128, ICOLS], BF16, tag="q", name="q")
                HC = 18 * 176
                for h in range(2):
                    lo, hi = h * HC, (h + 1) * HC
                    mb = mf[:].unsqueeze(-2).broadcast_to([128, 18, 176])
                    nc.vector._custom_dve(
                        snap,
                        out=q[:, lo:hi].rearrange("p (s g) -> p s g", g=176),
                        in0=xb[:, lo:hi].rearrange("p (s g) -> p s g", g=176),
                        in1=mb)
                    nc.scalar.dma_start(qx[j, :, lo:hi], q[:, lo:hi])

            # pair-batch so builtin<->custom DVE table swaps happen once per
            # pair instead of once per sample
            for j in range(0, 4, 2):
                a0 = absmax_phase(j)
                a1 = absmax_phase(j + 1)
                snap_phase(j, *a0)
                snap_phase(j + 1, *a1)
    nc.compile()
    return nc


def build_conv():
    nc = bacc.Bacc(None)
    qx4 = nc.declare_dram_parameter("qx4", [4, 128, T], BF16, isOutput=False)
    wblk = nc.declare_dram_parameter("wblk", [128, 6 * 128], BF16, isOutput=False)
    bias2 = nc.declare_dram_parameter("bias2", [128], F32, isOutput=False)
    out = nc.declare_dram_parameter("out", [4, 128, 6272], BF16, isOutput=True)

    # per-block max col = 904*(tb+3) + 1018; chunk loads gate block starts
    XCHUNKS = [0, 3760, 7360, 10976, T]

    with tile.TileContext(nc) as tc:
        with ExitStack() as ctx:
            consts = ctx.enter_context(tc.tile_pool(name="consts", bufs=1))
            xpool = ctx.enter_context(tc.tile_pool(name="x", bufs=2))
            opool = ctx.enter_context(tc.tile_pool(name="o", bufs=2))
            psum = ctx.enter_context(tc.tile_pool(name="ps", bufs=2,
                                                  space="PSUM"))

            wsb = consts.tile([128, 6 * 128], BF16)
            nc.sync.dma_start(wsb[:], wblk[:])
            bias_sb = consts.tile([128, 1], F32)
            nc.sync.dma_start(bias_sb[:], bias2[:, None])

            # PE warmup: dummy matmuls on a zeroed tile while the first input
            # chunks load, so HAM reaches 8/8 before the real stream starts
            warm = consts.tile([128, 512], BF16)
            nc.gpsimd.memset(warm[:], 0.0)
            wps = psum.tile([128, 512], F32, tag="ps0", name="wps")
            for w in range(10):
                nc.tensor.matmul(wps[:], warm[:, 0:128], warm[:],
                                 start=(w == 0), stop=(w == 9))

            for s in range(4):
                xt = xpool.tile([128, T], BF16, tag="xt")
                for a, b in zip(XCHUNKS, XCHUNKS[1:]):
                    nc.sync.dma_start(xt[:, a:b], qx4[s, :, a:b])
                osb = opool.tile([128, 6272], BF16, tag="osb")
                for tb in range(0, 14, 4):
                    nt = min(4, 14 - tb)
                    pss = [psum.tile([128, 512], F32, tag=f"ps{i}",
                                     name=f"ps{i}") for i in range(nt)]
                    for ci in range(6):
                        ch, dw = divmod(ci, 3)
                        lhs = wsb[:, ci * 128:(ci + 1) * 128]
                        for i in range(nt):
                            t = tb + i
                            base = 904 * t + 226 * ch + dw
                            rhs = xt[:, base:base + 904].rearrange(
                                "p (j u) -> p j u", u=226)[:, :, 0:113]
                            nc.tensor.matmul(pss[i][:, 0:452], lhs, rhs,
                                             start=(ci == 0), stop=(ci == 5))
                    for i in range(nt):
                        t = tb + i
                        nc.vector.tensor_scalar(
                            osb[:, t * 448:(t + 1) * 448].rearrange(
                                "p (j w) -> p j w", j=4),
                            pss[i][:, 0:452].rearrange(
                                "p (j u) -> p j u", j=4)[:, :, 1:113],
                            bias_sb[:, 0:1], None, op0=mybir.AluOpType.add)
                    if tb == 4:
                        nc.scalar.dma_start(out[s, :, 0:3584],
                                            osb[:, 0:3584])
                    elif tb == 12:
                        nc.scalar.dma_start(out[s, :, 3584:5376],
                                            osb[:, 3584:5376])
                nc.scalar.dma_start(out[s, :, 5376:6272], osb[:, 5376:6272])
    nc.compile()
    return nc


def _bfp_quantize_host(x):
    """Exact numpy replication of reference bfp_quantize (f32 semantics)."""
    flat = x.reshape(-1).astype(np.float32)
    n = flat.shape[0]
    pad = (-n) % GS
    f = np.concatenate([flat, np.zeros(pad, np.float32)]).reshape(-1, GS)
    m = np.max(np.abs(f), axis=1, keepdims=True).astype(np.float32)
    safe = np.where(m > 0, m, np.ones_like(m))
    e = np.floor(np.log2(safe)).astype(np.float32)
    scale = np.exp2(e - 7).astype(np.float32)
    q = (np.round(f / scale) * scale).astype(np.float32)
    q = np.where(m > 0, q, np.zeros_like(q))
    return q.reshape(-1)[:n].reshape(x.shape)


def _pack_weights(weight, bias):
    """wblk6 [128, 768] bf16 + bias128 [128] f32 (host-exact BFP quant)."""
    wq = _bfp_quantize_host(np.asarray(weight, np.float32))
    wb = np.zeros((128, 6, 128), np.float32)
    for ci in range(6):
        klow = -1 if ci < 3 else 1
        dw = ci % 3
        for ki in range(2):
            for rho in range(2):
                dh = (klow + ki) - rho + 1
                if 0 <= dh <= 2:
                    wb[64 * ki:64 * ki + 64, ci, 64 * rho:64 * rho + 64] = \
                        wq[:, :, dh, dw].T
    bias128 = np.concatenate([np.asarray(bias, np.float32)] * 2)
    return wb.reshape(128, 768).astype(NPBF16), bias128


def _shard_inputs(x):
    """Per-core bf16 group-aligned interleaved windows + per-sample phases."""
    xf = np.concatenate([np.asarray(x, np.float32).reshape(-1),
                         np.zeros(QWIN, np.float32)])
    xb = xf.astype(NPBF16)
    in_maps = []
    pres = []
    for k in range(N_CORES):
        core_pre = []
        xin = np.zeros((4, 128, 36, 176), NPBF16)
        for j in range(4):
            start = (4 * k + j) * SAMPLE
            g0 = (start // GS) * GS
            core_pre.append(start - g0)
            xin[j, :, :, 0:GPP] = (xb[g0:g0 + QWIN]
                                   .reshape(128, GPP, GS).transpose(0, 2, 1))
        in_maps.append({"xin": xin.reshape(4, 128, ICOLS)})
        pres.append(core_pre)
    return in_maps, pres


def _pack_conv_inputs(qx, core_pre, wblk6, bias128):
    """qx [4,128,6300] bf16 (window layout) -> conv in_map for one core."""
    dup = np.zeros((4, 128, T), NPBF16)
    for j in range(4):
        pre = core_pre[j]
        qw = (np.asarray(qx[j]).reshape(128, GS, 176)[:, :, 0:GPP]
              .transpose(0, 2, 1).reshape(-1))
        qs = qw[pre:pre + SAMPLE].reshape(C, H, W)
        Bq = np.zeros((C, 114, 113), NPBF16)
        Bq[:, 1:113, 1:113] = qs
        dup[j, :64, 1:12883] = Bq.reshape(C, 12882)
    dup[:, 64:, :T - 113] = dup[:, :64, 113:]
    return {"qx4": dup, "wblk": wblk6, "bias2": bias128}


def _unpack_out(od):
    """[4,128,6272] bf16 partition-major -> [4,64,112,112] f32."""
    return np.asarray(od).reshape(4, 2, 64, 14, 4, 112) \
        .transpose(0, 2, 3, 4, 1, 5).reshape(4, C, H, W).astype(np.float32)


def kernel(x, weight, bias):
    from concourse.bass_utils import run_bass_kernel_spmd

    if "quant" not in _cache:
        _cache["quant"] = build_quant()
    if "conv" not in _cache:
        _cache["conv"] = build_conv()

    core_ids = list(range(N_CORES))
    trace = _trace_enabled()
    if trace:
        _install_trace_shim()

    in_maps, pres = _shard_inputs(x)
    resA = run_bass_kernel_spmd(_cache["quant"], in_maps, core_ids, trace=trace)
    last_exec_ns["quant"] = resA.exec_time_ns
    last_results["quant"] = resA

    wblk6, bias128 = _pack_weights(weight, bias)
    in_maps_b = [
        _pack_conv_inputs(resA.results[k]["qx"], pres[k], wblk6, bias128)
        for k in range(N_CORES)
    ]
    resB = run_bass_kernel_spmd(_cache["conv"], in_maps_b, core_ids, trace=trace)
    last_exec_ns["conv"] = resB.exec_time_ns
    last_results["conv"] = resB

    out = np.concatenate(
        [_unpack_out(resB.results[k]["out"]) for k in range(N_CORES)], axis=0)
    return out


# revision 19
# speedup vs baseline: 1.1406x; 1.1406x over previous
"""BFP-quantized 3x3 conv (nn_BFConv2d) on 8 TRN2 NeuronCores.

Two-invocation structure (the BFP group grid is global over the flat
tensor, so each sample's quantized slab starts at a per-(sample,core)
phase pre = start mod 36; re-chunking by pre must happen host-side,
forcing quantize and conv into separate NEFF executions):

  inv1 QUANT: per core, 4 samples. Host supplies bf16-cast (RNE)
    group-aligned windows [128, 6300]. GpSimd computes per-group abs-max
    (groups are bf16 so the max is exact and its exponent equals the f32
    exponent), Vector applies the magic-number snap
        q = (x + M) - M,  M = 1.5 * 2^16 * exp2_bits(absmax)
    (round-half-even onto the BFP lattice; lattice points are <=9
    significant bits so bf16 holds q exactly). DMA in/out ~12.9MB/core.

  inv2 CONV: rho-paired matmul scheme at 75% PE utilization: PSUM
    partitions = 64 outch x 2 output-row-parity, K = 64 inch x 2
    input-row-offsets (dense 128x128 weights, host-built), 6 matmuls
    (2 K-chunks x 3 dw taps) of N=452 per 8-row tile. Input is a
    host-built duplicated padded layout [128, 12884] bf16 per sample
    (partitions 64-127 = same image shifted one row) with 113-strided
    rows sharing single zero pad columns. Output written bf16 in a
    partition-major layout, host de-interleaves rows and casts to f32.
"""

import os
import sys
from contextlib import ExitStack

import numpy as np

sys.path.insert(0, "/opt/trn_rl_repo")

import ml_dtypes  # noqa: E402
import concourse.bacc as bacc  # noqa: E402
import concourse.mybir as mybir  # noqa: E402
import concourse.tile as tile  # noqa: E402

F32 = mybir.dt.float32
BF16 = mybir.dt.bfloat16
I16 = mybir.dt.int16
NPBF16 = ml_dtypes.bfloat16

N_CORES = 8
B = 32                      # batch
C = 64                      # channels (in == out)
H = W = 112
SAMPLE = C * H * W          # 802816 elems per sample
GS = 36                     # BFP group size
GPP = 175                   # groups per partition in the quantize window
QCOLS = GPP * GS            # 6300
QWIN = 128 * QCOLS          # 806400 elems: covers a sample + phase slack
T = 12884                   # conv tile cols: 1 guard + 114*113 + 1 spare
MAGIC_MUL = 98304.0         # 1.5 * 2^16: exp2(e)*this == 1.5*2^23*2^(e-7)

_cache = {}
last_exec_ns = {}
last_results = {}


def _ensure_snap_op():
    """Register a custom DVE op BFP_SNAP_ANT: out = (in0 + in1) - in1."""
    import concourse.dve_ops as dops
    if getattr(dops, "_BFP_SNAP_ANT", None) is not None:
        return dops._BFP_SNAP_ANT
    from concourse.dve_spec import Spec, Src0, Src1, lower as spec_lower
    from concourse.dve_uop import DveOpSpec

    def _snap_ref(in0, in1, s0, s1, imm2):
        a = in0.astype(np.float32)
        b = np.broadcast_to(in1.astype(np.float32), in1.shape).reshape(a.shape)
        return (a + b) - b

    spec = Spec(body=(Src0 + Src1) - Src1, reference=_snap_ref)
    op = dops.DveOp("BFP_SNAP_ANT", spec, subdim=False, uops_sha={})
    idx = max(dops._SUB_OPCODE_FOR_NAME.values()) + 1
    assert idx < 0x20
    dops.OPS.append(op)
    dops.CUSTOM_DVE_SPECS["BFP_SNAP_ANT"] = spec
    dops._SUB_OPCODE_FOR_NAME["BFP_SNAP_ANT"] = idx
    for ver in ("v3", "v4"):
        try:
            s = DveOpSpec(name=op.name, opcode=idx,
                          uops=spec_lower(spec, ver=ver), rd1_en=True)
            op.uops_sha[ver] = s.sha(ver)
        except Exception:
            pass
    dops._BFP_SNAP_ANT = op
    return op


def _trace_enabled():
    return os.environ.get("BFP_TRACE") == "1"


def _install_trace_shim():
    """Provide antenv.axon_hooks (NTFF profiling hook) if the image lacks it."""
    import types
    import ctypes
    import contextlib
    try:
        from antenv.axon_hooks import get_axon_ntff_profile_hook  # noqa: F401
        return
    except ImportError:
        pass
    so_path = "/opt/axon/libaxon_pjrt.so"
    if not os.path.exists(so_path):
        return
    lib = ctypes.CDLL(so_path)
    if not hasattr(lib, "axon_start_nrt_profile"):
        return
    lib.axon_start_nrt_profile.argtypes = [ctypes.POINTER(ctypes.c_int64),
                                           ctypes.c_size_t]
    lib.axon_start_nrt_profile.restype = ctypes.c_int64
    lib.axon_stop_nrt_profile.argtypes = [ctypes.c_char_p]
    lib.axon_stop_nrt_profile.restype = ctypes.c_int64

    @contextlib.contextmanager
    def _hook(output_dir, device_ids):
        import jax
        jax.devices()
        if device_ids:
            ids = (ctypes.c_int64 * len(device_ids))(*device_ids)
            rc = lib.axon_start_nrt_profile(ids, len(device_ids))
        else:
            rc = lib.axon_start_nrt_profile(None, 0)
        if rc != 0:
            raise RuntimeError(f"axon_start_nrt_profile rc={rc}")
        try:
            yield
        finally:
            n = lib.axon_stop_nrt_profile(str(output_dir).encode())
            print(f"profile: {n} ntff file(s) -> {output_dir}", file=sys.stderr)

    mod = types.ModuleType("antenv.axon_hooks")
    state = {"hook": _hook}
    mod.get_axon_ntff_profile_hook = lambda: state["hook"]
    mod.set_axon_ntff_profile_hook = lambda h: state.update(hook=h)
    sys.modules["antenv.axon_hooks"] = mod
    import antenv
    antenv.axon_hooks = mod
    from concourse import bass_utils as bu
    bu.upload_artifacts = lambda d: str(d)  # no egress from this container


ICOLS = 36 * 176            # 6336: interleaved window, col = s*176 + g


def build_quant():
    """Interleaved-layout quant: host delivers windows as [s:36, g:176]
    (col = s*176 + g, g=175 is zero pad) so the per-group abs-max becomes a
    cascade of contiguous builtin tensor_tensor max ops (2x-eligible), with
    abs on ScalarE and the magic snap as a custom DVE op."""
    snap = _ensure_snap_op()
    nc = bacc.Bacc(None)
    xin = nc.declare_dram_parameter("xin", [4, 128, ICOLS], BF16, isOutput=False)
    qx = nc.declare_dram_parameter("qx", [4, 128, ICOLS], BF16, isOutput=True)

    MAX = mybir.AluOpType.max
    with tile.TileContext(nc) as tc:
        with ExitStack() as ctx:
            xpool = ctx.enter_context(tc.tile_pool(name="xp", bufs=3))
            apool = ctx.enter_context(tc.tile_pool(name="ap", bufs=2))
            qpool = ctx.enter_context(tc.tile_pool(name="qp", bufs=2))
            spool = ctx.enter_context(tc.tile_pool(name="small", bufs=2))

            def absmax_phase(j):
                xb = xpool.tile([128, ICOLS], BF16, tag="xb", name="xb")
                nc.sync.dma_start(xb[:], xin[j])
                xa = apool.tile([128, ICOLS], BF16, tag="xa", name="xa")
                nc.scalar.activation(xa[:], xb[:],
                                     mybir.ActivationFunctionType.Abs)
                u1 = spool.tile([128, 2816], BF16, tag="u1", name="u1")
                nc.vector.tensor_tensor(u1[:], xa[:, 0:2816], xa[:, 2816:5632], MAX)
                u2 = spool.tile([128, 1408], BF16, tag="u2", name="u2")
                nc.vector.tensor_tensor(u2[:], u1[:, 0:1408], u1[:, 1408:2816], MAX)
                u3 = spool.tile([128, 704], BF16, tag="u3", name="u3")
                nc.vector.tensor_tensor(u3[:], u2[:, 0:704], u2[:, 704:1408], MAX)
                u4 = spool.tile([128, 704], BF16, tag="u4", name="u4")
                nc.vector.tensor_tensor(u4[:], u3[:, 0:704], xa[:, 5632:6336], MAX)
                u5 = spool.tile([128, 352], BF16, tag="u5", name="u5")
                nc.vector.tensor_tensor(u5[:], u4[:, 0:352], u4[:, 352:704], MAX)
                m = spool.tile([128, 176], BF16, tag="m", name="m")
                nc.vector.tensor_tensor(m[:], u5[:, 0:176], u5[:, 176:352], MAX)
                mi = spool.tile([128, 176], I16, tag="mi", name="mi")
                nc.vector.tensor_scalar(mi[:], m[:].bitcast(I16), 0x7F80, None,
                                        op0=mybir.AluOpType.bitwise_and)
                mf = spool.tile([128, 176], BF16, tag="mf", name="mf")
                nc.vector.tensor_scalar(mf[:], mi[:].bitcast(BF16), MAGIC_MUL,
                                        None, op0=mybir.AluOpType.mult)
                return xb, mf

            def snap_phase(j, xb, mf):
                q = qpool.tile([128, ICOLS], BF16, tag="q", name="q")
                mb = mf[:].unsqueeze(-2).broadcast_to([128, GS, 176])
                nc.vector._custom_dve(
                    snap, out=q[:].rearrange("p (s g) -> p s g", g=176),
                    in0=xb[:].rearrange("p (s g) -> p s g", g=176), in1=mb)
                nc.scalar.dma_start(qx[j], q[:])

            # pair-batch so builtin<->custom DVE table swaps happen once per
            # pair instead of once per sample
            for j in range(0, 4, 2):
                a0 = absmax_phase(j)
                a1 = absmax_phase(j + 1)
                snap_phase(j, *a0)
                snap_phase(j + 1, *a1)
    nc.compile()
    return nc


def build_conv():
    nc = bacc.Bacc(None)
    qx4 = nc.declare_dram_parameter("qx4", [4, 128, T], BF16, isOutput=False)
    wblk = nc.declare_dram_parameter("wblk", [128, 6 * 128], BF16, isOutput=False)
    bias2 = nc.declare_dram_parameter("bias2", [128], F32, isOutput=False)
    out = nc.declare_dram_parameter("out", [4, 128, 6272], BF16, isOutput=True)

    # per-block max col = 904*(tb+3) + 1018; chunk loads gate block starts
    XCHUNKS = [0, 3760, 7360, 10976, T]

    with tile.TileContext(nc) as tc:
        with ExitStack() as ctx:
            consts = ctx.enter_context(tc.tile_pool(name="consts", bufs=1))
            xpool = ctx.enter_context(tc.tile_pool(name="x", bufs=2))
            opool = ctx.enter_context(tc.tile_pool(name="o", bufs=2))
            psum = ctx.enter_context(tc.tile_pool(name="ps", bufs=2,
                                                  space="PSUM"))

            wsb = consts.tile([128, 6 * 128], BF16)
            nc.sync.dma_start(wsb[:], wblk[:])
            bias_sb = consts.tile([128, 1], F32)
            nc.sync.dma_start(bias_sb[:], bias2[:, None])

            # PE warmup: dummy matmuls on a zeroed tile while the first input
            # chunks load, so HAM reaches 8/8 before the real stream starts
            warm = consts.tile([128, 512], BF16)
            nc.gpsimd.memset(warm[:], 0.0)
            wps = psum.tile([128, 512], F32, tag="ps0", name="wps")
            for w in range(10):
                nc.tensor.matmul(wps[:], warm[:, 0:128], warm[:],
                                 start=(w == 0), stop=(w == 9))

            for s in range(4):
                xt = xpool.tile([128, T], BF16, tag="xt")
                for a, b in zip(XCHUNKS, XCHUNKS[1:]):
                    nc.sync.dma_start(xt[:, a:b], qx4[s, :, a:b])
                osb = opool.tile([128, 6272], BF16, tag="osb")
                for tb in range(0, 14, 4):
                    nt = min(4, 14 - tb)
                    pss = [psum.tile([128, 512], F32, tag=f"ps{i}",
                                     name=f"ps{i}") for i in range(nt)]
                    for ci in range(6):
                        ch, dw = divmod(ci, 3)
                        lhs = wsb[:, ci * 128:(ci + 1) * 128]
                        for i in range(nt):
                            t = tb + i
                            base = 904 * t + 226 * ch + dw
                            rhs = xt[:, base:base + 904].rearrange(
                                "p (j u) -> p j u", u=226)[:, :, 0:113]
                            nc.tensor.matmul(pss[i][:, 0:452], lhs, rhs,
                                             start=(ci == 0), stop=(ci == 5))
                    for i in range(nt):
                        t = tb + i
                        nc.vector.tensor_scalar(
                            osb[:, t * 448:(t + 1) * 448].rearrange(
                                "p (j w) -> p j w", j=4),
                            pss[i][:, 0:452].rearrange(
                                "p (j u) -> p j u", j=4)[:, :, 1:113],
                            bias_sb[:, 0:1], None, op0=mybir.AluOpType.add)
                    if tb == 4:
                        nc.scalar.dma_start(out[s, :, 0:3584],
                                            osb[:, 0:3584])
                    elif tb == 12:
                        nc.scalar.dma_start(out[s, :, 3584:5376],
                                            osb[:, 3584:5376])
                nc.scalar.dma_start(out[s, :, 5376:6272], osb[:, 5376:6272])
    nc.compile()
    return nc


def _bfp_quantize_host(x):
    """Exact numpy replication of reference bfp_quantize (f32 semantics)."""
    flat = x.reshape(-1).astype(np.float32)
    n = flat.shape[0]
    pad = (-n) % GS
    f = np.concatenate([flat, np.zeros(pad, np.float32)]).reshape(-1, GS)
    m = np.max(np.abs(f), axis=1, keepdims=True).astype(np.float32)
    safe = np.where(m > 0, m, np.ones_like(m))
    e = np.floor(np.log2(safe)).astype(np.float32)
    scale = np.exp2(e - 7).astype(np.float32)
    q = (np.round(f / scale) * scale).astype(np.float32)
    q = np.where(m > 0, q, np.zeros_like(q))
    return q.reshape(-1)[:n].reshape(x.shape)


def _pack_weights(weight, bias):
    """wblk6 [128, 768] bf16 + bias128 [128] f32 (host-exact BFP quant)."""
    wq = _bfp_quantize_host(np.asarray(weight, np.float32))
    wb = np.zeros((128, 6, 128), np.float32)
    for ci in range(6):
        klow = -1 if ci < 3 else 1
        dw = ci % 3
        for ki in range(2):
            for rho in range(2):
                dh = (klow + ki) - rho + 1
                if 0 <= dh <= 2:
                    wb[64 * ki:64 * ki + 64, ci, 64 * rho:64 * rho + 64] = \
                        wq[:, :, dh, dw].T
    bias128 = np.concatenate([np.asarray(bias, np.float32)] * 2)
    return wb.reshape(128, 768).astype(NPBF16), bias128


def _shard_inputs(x):
    """Per-core bf16 group-aligned interleaved windows + per-sample phases."""
    xf = np.concatenate([np.asarray(x, np.float32).reshape(-1),
                         np.zeros(QWIN, np.float32)])
    xb = xf.astype(NPBF16)
    in_maps = []
    pres = []
    for k in range(N_CORES):
        core_pre = []
        xin = np.zeros((4, 128, 36, 176), NPBF16)
        for j in range(4):
            start = (4 * k + j) * SAMPLE
            g0 = (start // GS) * GS
            core_pre.append(start - g0)
            xin[j, :, :, 0:GPP] = (xb[g0:g0 + QWIN]
                                   .reshape(128, GPP, GS).transpose(0, 2, 1))
        in_maps.append({"xin": xin.reshape(4, 128, ICOLS)})
        pres.append(core_pre)
    return in_maps, pres


def _pack_conv_inputs(qx, core_pre, wblk6, bias128):
    """qx [4,128,6300] bf16 (window layout) -> conv in_map for one core."""
    dup = np.zeros((4, 128, T), NPBF16)
    for j in range(4):
        pre = core_pre[j]
        qw = (np.asarray(qx[j]).reshape(128, GS, 176)[:, :, 0:GPP]
              .transpose(0, 2, 1).reshape(-1))
        qs = qw[pre:pre + SAMPLE].reshape(C, H, W)
        Bq = np.zeros((C, 114, 113), NPBF16)
        Bq[:, 1:113, 1:113] = qs
        dup[j, :64, 1:12883] = Bq.reshape(C, 12882)
    dup[:, 64:, :T - 113] = dup[:, :64, 113:]
    return {"qx4": dup, "wblk": wblk6, "bias2": bias128}


def _unpack_out(od):
    """[4,128,6272] bf16 partition-major -> [4,64,112,112] f32."""
    return np.asarray(od).reshape(4, 2, 64, 14, 4, 112) \
        .transpose(0, 2, 3, 4, 1, 5).reshape(4, C, H, W).astype(np.float32)


def kernel(x, weight, bias):
    from concourse.bass_utils import run_bass_kernel_spmd

    if "quant" not in _cache:
        _cache["quant"] = build_quant()
    if "conv" not in _cache:
        _cache["conv"] = build_conv()

    core_ids = list(range(N_CORES))
    trace = _trace_enabled()
    if trace:
        _install_trace_shim()

    in_maps, pres = _shard_inputs(x)
    resA = run_bass_kernel_spmd(_cache["quant"], in_maps, core_ids, trace=trace)
    last_exec_ns["quant"] = resA.exec_time_ns
    last_results["quant"] = resA

    wblk6, bias128 = _pack_weights(weight, bias)
    in_maps_b = [
        _pack_conv_inputs(resA.results[k]["qx"], pres[k], wblk6, bias128)
        for k in range(N_CORES)
    ]
    resB = run_bass_kernel_spmd(_cache["conv"], in_maps_b, core_ids, trace=trace)
    last_exec_ns["conv"] = resB.exec_time_ns
    last_results["conv"] = resB

    out = np.concatenate(
        [_unpack_out(resB.results[k]["out"]) for k in range(N_CORES)], axis=0)
    return out
